# revision 19
# baseline (speedup 1.0000x reference)
"""CP-decomposed weight reconstruction on 8 trn2 NeuronCores.

Problem: out[a,b,c,d] = sum_e core[e]*fa[a,e]*fb[b,e]*fc[c,e]*fd[d,e]
(complex64), shapes a=64 b=64 c=128 d=65 e=32.  Output ~273 MB — the
kernel is output-write bound (memory regime).

Strategy (per core, a sharded 8 ways -> a_local=8, M = a_local*b = 512):
  - Host folds core into fa, forms A[m,e] = (fa*core)[a,e]*fb[b,e] and
    B[n,e] = fc[c,e]*fd[d,e]  (m=(a_l,b), n=(c,d); both tiny).
  - Complex contraction out[m,n] = sum_e A[m,e]*B[n,e] is ONE real K=64
    GEMM per tile: stack Re/Im of A along K for lhsT, and build rhs so
    output columns interleave (re,im) pairs -> the fp32 result written to
    HBM IS the complex64 layout, no post-processing on device.
       lhsT[e,   m] = Re A[m,e]      rhs[e,   2n] = Re B[n,e]   rhs[e,   2n+1] = Im B[n,e]
       lhsT[32+e,m] = Im A[m,e]      rhs[32+e,2n] = -Im B[n,e]  rhs[32+e,2n+1] = Re B[n,e]
  - rhs is [64, 16640] f32; packed as [128, 8320] (column halves on
    partition halves) so its DMA uses all 16 ports; matmuls for the right
    half read partitions 64:128 with lhsT duplicated on both halves.
    Adjacent left/right matmuls sit on disjoint PE row-groups and execute
    CONCURRENTLY (~2x PE throughput — measured), which hides the fp32
    2-pass matmul cost (~72 us PE) under the store stream.
  - Loop: 4 m-strips of 128 rows; per strip 16 psum pair-tiles [128,1024]
    (2 banks) + 2 remainder tiles; copies alternate DVE/ACT 50/50
    (combined ~920 GB/s evacuation) into a [128, 16640] staging strip;
    stores go out on alternating HWDGE rings; strips 0-2 store as
    quarters in COMPLETION order (quarters [0:4160]/[8320:12480] finish
    at mid-strip thanks to the pair interleave, so the store stream
    never starves while the still-cold PE finishes the strip), strip 3
    stores whole at the peak ~411 GB/s rate.  Measured ~110 us/core vs
    a ~103 us traffic floor (38.6 MB/core) plus ~16 us fixed
    preamble/drain overhead; run-to-run variance +-5-8 us from paired-
    core HBM stack contention.
"""

import os
import sys

sys.path.insert(0, "/opt/trn_rl_repo")

import numpy as np

import concourse.bass as bass
import concourse.bacc as bacc
import concourse.mybir as mybir
import concourse.tile as tile
from concourse.bass_utils import run_bass_kernel_spmd

# ---- problem constants (hardcoded per contract) ----
RANK = 32
A, B, C, D = 64, 64, 128, 65
N_CORES = 8
A_LOC = A // N_CORES          # 8 a-rows per core
M = A_LOC * B                 # 512 output rows per core
NC_F32 = C * D * 2            # 16640 f32 output cols (re/im interleaved)
N_HALF = NC_F32 // 2          # 8320
K = 2 * RANK                  # 64 (Re and Im stacked)
STRIP = 128                   # psum/output partition strip
N_STRIPS = M // STRIP         # 4
PAIR = 1024                   # psum pair tile (2 banks)
FULL = 512                    # matmul free size (one psum bank of f32)
N_PAIRS = N_HALF // PAIR      # 8; remainder 128
REM = N_HALF - N_PAIRS * PAIR  # 128

VARIANT = os.environ.get("BASS_KERNEL_VARIANT", "f32")  # f32 | f32r
TRACE = os.environ.get("BASS_KERNEL_TRACE", "0") == "1"
WARMUP = int(os.environ.get("BASS_KERNEL_WARMUP", "0"))
OUT_SPLIT = os.environ.get("BASS_KERNEL_OUT_SPLIT", "ordq")  # ordq | full

LAST_RESULTS = None  # BassKernelResults of the last run (for test.py)

_PROGRAM = None

_AXON_SO = "/opt/axon/libaxon_pjrt.so"


def _install_trace_shim():
    """This image's antenv lacks axon_hooks, so run_bass_kernel_spmd's
    trace path can't find the NTFF profile hook. Recreate it: drive NRT
    profiling via ctypes into libaxon_pjrt.so (same contract as
    trn_agent_boot), and stub out the S3 artifact upload."""
    import contextlib
    import ctypes
    import types

    import antenv
    import concourse.bass_utils as bu

    bu.upload_artifacts = lambda tmpdir: tmpdir  # no S3 here

    if "antenv.axon_hooks" in sys.modules:
        return
    try:
        lib = ctypes.CDLL(_AXON_SO)
        lib.axon_start_nrt_profile.argtypes = [
            ctypes.POINTER(ctypes.c_int64), ctypes.c_size_t]
        lib.axon_start_nrt_profile.restype = ctypes.c_int64
        lib.axon_stop_nrt_profile.argtypes = [ctypes.c_char_p]
        lib.axon_stop_nrt_profile.restype = ctypes.c_int64
    except OSError:
        lib = None

    @contextlib.contextmanager
    def _hook(output_dir, device_ids):
        import jax
        jax.devices()
        if device_ids:
            ids = (ctypes.c_int64 * len(device_ids))(*device_ids)
            rc = lib.axon_start_nrt_profile(ids, len(device_ids))
        else:
            rc = lib.axon_start_nrt_profile(None, 0)
        if rc != 0:
            raise RuntimeError(f"axon_start_nrt_profile rc={rc}")
        try:
            yield
        finally:
            n = lib.axon_stop_nrt_profile(str(output_dir).encode())
            print(f"ntff profile: {n} file(s) written to {output_dir}",
                  file=sys.stderr)

    mod = types.ModuleType("antenv.axon_hooks")
    mod.get_axon_ntff_profile_hook = (lambda: _hook) if lib else (lambda: None)
    mod.set_axon_ntff_profile_hook = lambda h: None
    sys.modules["antenv.axon_hooks"] = mod
    antenv.axon_hooks = mod


def _mm_dtype():
    return mybir.dt.float32r if VARIANT == "f32r" else mybir.dt.float32


def _build_program_bf16():
    """bf16 hi/lo-split variant: out = Lh.Rh + Lh.Rl + Ll.Rh computed as
    two accumulating bf16 matmuls per tile:
      MM1 K=128: lhsT=[Lh;Lh], rhs=[Rh;Rl]   MM2 K=64: lhsT=Ll, rhs=Rh
    Input layout (single [128, 1024+16640] bf16 tensor, one DMA):
      cols 0:512      = lhsT1 = [Lh; Lh]      (K=128 x M=512)
      cols 512:1024   = lhsT2 = [Ll; zeros]   (rows 0:64 used)
      cols 1024:17664 = rhs1  = [Rh; Rl]      (K=128 x N=16640)
    """
    nc = bacc.Bacc("TRN2", target_bir_lowering=False, debug=False,
                   enable_asserts=False)
    bf16 = mybir.dt.bfloat16
    f32 = mybir.dt.float32
    W_IN = 2 * M + NC_F32  # 17664

    inp_d = nc.dram_tensor("inp", [128, W_IN], bf16, kind="ExternalInput").ap()
    out_d = nc.dram_tensor("out", [M, NC_F32], f32, kind="ExternalOutput").ap()

    with tile.TileContext(nc) as tc:
        with (
            tc.tile_pool(name="const", bufs=1) as const_pool,
            tc.tile_pool(name="stage", bufs=2) as stage_pool,
            tc.tile_pool(name="psum", bufs=4, space="PSUM") as psum_pool,
        ):
            inp_s = const_pool.tile([128, W_IN], bf16)
            nc.sync.dma_start(out=inp_s[:], in_=inp_d[:])
            lhsT1 = inp_s[:, 0:M]
            lhsT2 = inp_s[0:64, M:2 * M]
            rhs1 = inp_s[:, 2 * M:]
            rhs2 = inp_s[0:64, 2 * M:]

            copy_ctr = 0
            for s in range(N_STRIPS):
                staging = stage_pool.tile([128, NC_F32], f32, tag="staging")
                l1 = lhsT1[:, s * STRIP:(s + 1) * STRIP]
                l2 = lhsT2[:, s * STRIP:(s + 1) * STRIP]

                def do_block(n0, width):
                    nonlocal copy_ctr
                    ps = psum_pool.tile([128, PAIR], f32, tag="ps")
                    off = 0
                    while off < width:
                        w = min(FULL, width - off)
                        nc.tensor.matmul(
                            ps[:, off:off + w], l1,
                            rhs1[:, n0 + off:n0 + off + w],
                            start=True, stop=False)
                        nc.tensor.matmul(
                            ps[:, off:off + w], l2,
                            rhs2[:, n0 + off:n0 + off + w],
                            start=False, stop=True)
                        off += w
                    dst = staging[:, n0:n0 + width]
                    if copy_ctr % 4 == 3:
                        nc.scalar.copy(out=dst, in_=ps[:, 0:width])
                    else:
                        nc.vector.tensor_copy(out=dst, in_=ps[:, 0:width])
                    copy_ctr += 1

                for q in range(NC_F32 // PAIR):      # 16 pairs
                    do_block(q * PAIR, PAIR)
                do_block((NC_F32 // PAIR) * PAIR, NC_F32 % PAIR)  # 256 rem

                nc.sync.dma_start(
                    out=out_d[s * STRIP:(s + 1) * STRIP, :],
                    in_=staging[:],
                )
    nc.compile()
    return nc


W_UV = 4 * 64 + 4 * 65        # 516 cols of U/V outer-product factors
W_DEV = W_UV + M              # devrhs input width: U/V + lhsT


def _build_program_devrhs():
    """a-shard f32 variant with the rhs BUILT ON DEVICE instead of loaded:
    rhs[p, 2n'+t] = U[p,c']*V[p,d] + U2[p,c']*V2[p,d] (outer products over
    (c', d) with per-partition-group sign/component selection baked into
    tiny host-prepared U [128,64] / V [128,65] tensors).  Cuts the input
    DMA from 4.52 MB to 0.53 MB -> ~12 us less HBM traffic (the kernel is
    HBM-bound at ~358 GB/s/core).  Even-column ops on DVE, odd on GPSIMD
    (otherwise idle), chunked 8x520 so matmuls start after chunk 0."""
    nc = bacc.Bacc("TRN2", target_bir_lowering=False, debug=False,
                   enable_asserts=False)
    f32 = mybir.dt.float32
    AOP = mybir.AluOpType

    inp_d = nc.dram_tensor("inp", [128, W_DEV], f32,
                           kind="ExternalInput").ap()
    out_d = nc.dram_tensor("out", [M, NC_F32], f32,
                           kind="ExternalOutput").ap()

    with tile.TileContext(nc) as tc:
        with (
            tc.tile_pool(name="const", bufs=1) as const_pool,
            tc.tile_pool(name="scratch", bufs=2) as scratch_pool,
            tc.tile_pool(name="stage", bufs=2) as stage_pool,
            tc.tile_pool(name="psum", bufs=4, space="PSUM") as psum_pool,
        ):
            inp_s = const_pool.tile([128, W_DEV], f32)
            rhs_t = const_pool.tile([128, N_HALF], f32)
            # U/V first so the rhs build starts ASAP; lhsT second (the
            # first matmul then depends on one DMA lane + compute sems)
            nc.scalar.dma_start(out=inp_s[:, 0:W_UV], in_=inp_d[:, 0:W_UV])
            nc.scalar.dma_start(out=inp_s[:, W_UV:W_DEV],
                                in_=inp_d[:, W_UV:W_DEV])
            lhsT_s = inp_s[:, W_UV:W_UV + M]

            def ubc(col0, k):
                return (inp_s[:, col0 + 8 * k:col0 + 8 * k + 8]
                        .unsqueeze(2).broadcast_to([128, 8, 65]))

            def vbc(col0):
                return (inp_s[:, col0:col0 + 65]
                        .unsqueeze(1).broadcast_to([128, 8, 65]))

            VE1b, VE2b = vbc(256), vbc(321)
            VO1b, VO2b = vbc(386), vbc(451)

            for k in range(8):
                blk = (rhs_t[:, 1040 * k:1040 * (k + 1)]
                       .rearrange("p (c d t) -> p c d t", c=8, d=65, t=2))
                ev, od = blk[:, :, :, 0], blk[:, :, :, 1]
                t1 = scratch_pool.tile([128, 520], f32, tag="t1")
                t2 = scratch_pool.tile([128, 520], f32, tag="t2")
                t1v = t1[:].rearrange("p (c d) -> p c d", c=8, d=65)
                t2v = t2[:].rearrange("p (c d) -> p c d", c=8, d=65)
                nc.vector.tensor_tensor(t1v, ubc(0, k), VE1b, AOP.mult)
                nc.vector.tensor_tensor(t2v, ubc(64, k), VE2b, AOP.mult)
                nc.vector.scalar_tensor_tensor(ev, t1v, 0.0, t2v,
                                               AOP.bypass, AOP.add)
                t3 = scratch_pool.tile([128, 520], f32, tag="t3")
                t4 = scratch_pool.tile([128, 520], f32, tag="t4")
                t3v = t3[:].rearrange("p (c d) -> p c d", c=8, d=65)
                t4v = t4[:].rearrange("p (c d) -> p c d", c=8, d=65)
                nc.gpsimd.tensor_tensor(t3v, ubc(128, k), VO1b, AOP.mult)
                nc.gpsimd.tensor_tensor(t4v, ubc(192, k), VO2b, AOP.mult)
                nc.gpsimd.tensor_tensor(od, t3v, t4v, AOP.add)

            copy_ctr = 0
            for s in range(N_STRIPS):
                staging = stage_pool.tile([128, NC_F32], f32, tag="staging")
                lhs_lo = lhsT_s[0:K, s * STRIP:(s + 1) * STRIP]
                lhs_hi = lhsT_s[64:64 + K, s * STRIP:(s + 1) * STRIP]

                def do_block(n0, width, h):
                    nonlocal copy_ctr
                    ps = psum_pool.tile([128, PAIR], f32, tag="ps")
                    lhs = lhs_lo if h == 0 else lhs_hi
                    rk = rhs_t[0:K] if h == 0 else rhs_t[64:64 + K]
                    off = 0
                    while off < width:
                        w = min(FULL, width - off)
                        nc.tensor.matmul(
                            ps[:, off:off + w],
                            lhs,
                            rk[:, n0 + off:n0 + off + w],
                            start=True, stop=True,
                        )
                        off += w
                    dst = staging[:, h * N_HALF + n0: h * N_HALF + n0 + width]
                    if copy_ctr % 2 == 1:
                        nc.scalar.copy(out=dst, in_=ps[:, 0:width])
                    else:
                        nc.vector.tensor_copy(out=dst, in_=ps[:, 0:width])
                    copy_ctr += 1

                for q in range(N_PAIRS):
                    for h in (0, 1):
                        do_block(q * PAIR, PAIR, h)
                for h in (0, 1):
                    do_block(N_PAIRS * PAIR, REM, h)

                if s < 3:
                    pieces = [(0, 4160), (8320, 12480),
                              (4160, 8320), (12480, NC_F32)]
                else:
                    pieces = [(0, NC_F32)]
                for oi, (c0, c1) in enumerate(pieces):
                    eng = nc.sync if (s + oi) % 2 == 0 else nc.scalar
                    eng.dma_start(
                        out=out_d[s * STRIP:(s + 1) * STRIP, c0:c1],
                        in_=staging[:, c0:c1],
                    )
    nc.compile()
    return nc


W_U2 = 2 * C                  # 256: (U_e, U_o) interleaved per c
W_V2 = 2 * D                  # 130: (V_e, V_o) interleaved per d
W_K128 = W_U2 + W_V2 + M      # 898 input cols
K128_CHUNK = 8                # c's per build op -> 1040 dense cols
K128_NCHUNK = C // K128_CHUNK  # 16
K128_GPSIMD = os.environ.get("BASS_K128_GPSIMD", "1") == "1"


def _build_program_k128():
    """K=128 scheme: contraction dim = 4 groups x 32 ranks, so the PE's
    K-reduction performs the complex-arithmetic combines.  The rhs is then
    PURE PRODUCTS  rhs[k, 2(c*65+d)+t] = U2[k, 2c+t] * V2[k, 2d+t]  and
    since j = 130c + 2d + t is dense in (c,d,t), each build op writes a
    fully contiguous 1040-col block: one tensor_tensor mult per chunk, no
    adds, no scratch, no strided-write penalty (the devrhs variant showed
    stride-2 SBUF writes run ~4x slow).  Input drops to 0.46 MB.
    Matmuls run float32r (1 cyc/row at N>=256; fp32 at K=128 would be
    4 cyc/row with no row-group pairing possible = PE-bound) via bitcast
    of the f32-built rhs; psum stays f32."""
    nc = bacc.Bacc("TRN2", target_bir_lowering=False, debug=False,
                   enable_asserts=False)
    f32 = mybir.dt.float32
    f32r = mybir.dt.float32r
    AOP = mybir.AluOpType

    inp_uv_d = nc.dram_tensor("uv", [128, W_U2 + W_V2], f32,
                              kind="ExternalInput").ap()
    inp_l_d = nc.dram_tensor("lhs", [128, M], f32r,
                             kind="ExternalInput").ap()
    out_d = nc.dram_tensor("out", [M, NC_F32], f32,
                           kind="ExternalOutput").ap()

    with tile.TileContext(nc) as tc:
        with (
            tc.tile_pool(name="const", bufs=1) as const_pool,
            tc.tile_pool(name="stage", bufs=2) as stage_pool,
            tc.tile_pool(name="psum", bufs=4, space="PSUM") as psum_pool,
        ):
            uv_s = const_pool.tile([128, W_U2 + W_V2], f32)
            lhsT_s = const_pool.tile([128, M], f32r)
            rhs_t = const_pool.tile([128, NC_F32], f32r)
            # U2/V2 first so the build starts ASAP; lhsT second (first
            # matmul then waits on one DMA lane + build sems only)
            nc.scalar.dma_start(out=uv_s[:], in_=inp_uv_d[:])
            nc.scalar.dma_start(out=lhsT_s[:], in_=inp_l_d[:])

            # PE p-state warm-up while the input DMA is in flight (plain
            # f32 so no f32r-producer rounding rule applies)
            if WARMUP:
                warm = const_pool.tile([128, 128], f32)
                nc.gpsimd.memset(warm[:], 0.0)
                ps_w = psum_pool.tile([128, PAIR], f32, tag="ps")
                for _ in range(WARMUP):
                    nc.tensor.matmul(ps_w[:, 0:128], warm[:], warm[:],
                                     start=True, stop=True)

            v2b = (uv_s[:, W_U2:W_U2 + W_V2]
                   .rearrange("p (d t) -> p d t", d=D, t=2)
                   .unsqueeze(1).broadcast_to([128, K128_CHUNK, D, 2]))
            for k in range(K128_NCHUNK):
                u2b = (uv_s[:, 2 * K128_CHUNK * k:2 * K128_CHUNK * (k + 1)]
                       .rearrange("p (c t) -> p c t", c=K128_CHUNK, t=2)
                       .unsqueeze(2).broadcast_to([128, K128_CHUNK, D, 2]))
                dst = (rhs_t[:, 1040 * k:1040 * (k + 1)]
                       .rearrange("p (c d t) -> p c d t",
                                  c=K128_CHUNK, d=D, t=2))
                eng = nc.gpsimd if (K128_GPSIMD and k % 3 == 2) else nc.vector
                eng.tensor_tensor(dst, u2b, v2b, AOP.mult)

            copy_ctr = 0
            for s in range(N_STRIPS):
                staging = stage_pool.tile([128, NC_F32], f32, tag="staging")
                lhs = lhsT_s[:, s * STRIP:(s + 1) * STRIP]

                def do_block(n0, width):
                    nonlocal copy_ctr
                    ps = psum_pool.tile([128, PAIR], f32, tag="ps")
                    off = 0
                    while off < width:
                        w = min(FULL, width - off)
                        nc.tensor.matmul(
                            ps[:, off:off + w],
                            lhs,
                            rhs_t[:, n0 + off:n0 + off + w],
                            start=True, stop=True,
                        )
                        off += w
                    dst = staging[:, n0:n0 + width]
                    # strip 0: ACT only (DVE is busy building the rhs);
                    # later strips alternate DVE/ACT
                    if s == 0 or copy_ctr % 2 == 1:
                        nc.scalar.copy(out=dst, in_=ps[:, 0:width])
                    else:
                        nc.vector.tensor_copy(out=dst, in_=ps[:, 0:width])
                    copy_ctr += 1

                for q in range(NC_F32 // PAIR):          # 16 pairs
                    do_block(q * PAIR, PAIR)
                do_block((NC_F32 // PAIR) * PAIR, NC_F32 % PAIR)  # 256

                # stores in completion (left-to-right) order: strip 0 in
                # 1.06 MB eighths so the stream starts early, strips 1-2
                # quarters, strip 3 whole
                if s == 0:   # pair-aligned 1 MB pieces, 2.3 MB tail
                    pieces = [(i * 2048, (i + 1) * 2048) for i in range(7)]
                    pieces.append((14336, NC_F32))
                elif s < 3:  # pair-aligned 2 MB pieces
                    pieces = [(0, 4096), (4096, 8192), (8192, 12288),
                              (12288, NC_F32)]
                else:
                    pieces = [(0, NC_F32)]
                for oi, (c0, c1) in enumerate(pieces):
                    eng = nc.sync if (s + oi) % 2 == 0 else nc.scalar
                    eng.dma_start(
                        out=out_d[s * STRIP:(s + 1) * STRIP, c0:c1],
                        in_=staging[:, c0:c1],
                    )
    nc.compile()
    return nc


WSB = 2048                    # superblock column width (f32 cols)
NSB = 8                       # 7 x 2048 + final 2304
K128_SPLIT = os.environ.get("BASS_K128_SPLIT", "1") == "1"


def _build_program_k128b():
    """k128 with COLUMN-OUTER ordering: as soon as rhs chunks 2sb,2sb+1
    are built, compute that column block for ALL 4 strips and store each
    [128, wsb] piece immediately.  Production reaches the store stream
    ~4x sooner than strip-outer order (which serialized strip 0 behind
    the whole build and starved the DMA queues for ~25 us)."""
    nc = bacc.Bacc("TRN2", target_bir_lowering=False, debug=False,
                   enable_asserts=False)
    f32 = mybir.dt.float32
    f32r = mybir.dt.float32r
    AOP = mybir.AluOpType

    inp_uv_d = nc.dram_tensor("uv", [128, W_U2 + W_V2], f32,
                              kind="ExternalInput").ap()
    inp_l_d = nc.dram_tensor("lhs", [128, M], f32r,
                             kind="ExternalInput").ap()
    out_d = nc.dram_tensor("out", [M, NC_F32], f32,
                           kind="ExternalOutput").ap()

    with tile.TileContext(nc) as tc:
        with (
            tc.tile_pool(name="const", bufs=1) as const_pool,
            tc.tile_pool(name="stage", bufs=2) as stage_pool,
            tc.tile_pool(name="psum", bufs=4, space="PSUM") as psum_pool,
        ):
            uv_s = const_pool.tile([128, W_U2 + W_V2], f32)
            lhsT_s = const_pool.tile([128, M], f32r)
            rhs_t = const_pool.tile([128, NC_F32], f32r)
            nc.scalar.dma_start(out=uv_s[:], in_=inp_uv_d[:])
            nc.scalar.dma_start(out=lhsT_s[:], in_=inp_l_d[:])

            if WARMUP:
                warm = const_pool.tile([128, 128], f32)
                nc.gpsimd.memset(warm[:], 0.0)
                ps_w = psum_pool.tile([128, PAIR], f32, tag="ps")
                for _ in range(WARMUP):
                    nc.tensor.matmul(ps_w[:, 0:128], warm[:], warm[:],
                                     start=True, stop=True)

            v2b = (uv_s[:, W_U2:W_U2 + W_V2]
                   .rearrange("p (d t) -> p d t", d=D, t=2)
                   .unsqueeze(1).broadcast_to([128, K128_CHUNK, D, 2]))

            def build_chunk(k):
                u2b = (uv_s[:, 2 * K128_CHUNK * k:2 * K128_CHUNK * (k + 1)]
                       .rearrange("p (c t) -> p c t", c=K128_CHUNK, t=2)
                       .unsqueeze(2).broadcast_to([128, K128_CHUNK, D, 2]))
                dst = (rhs_t[:, 1040 * k:1040 * (k + 1)]
                       .rearrange("p (c d t) -> p c d t",
                                  c=K128_CHUNK, d=D, t=2))
                eng = nc.gpsimd if (K128_GPSIMD and k % 3 == 1) else nc.vector
                eng.tensor_tensor(dst, u2b, v2b, AOP.mult)

            # Row split across strips: SDMA engine 15 (serving partitions
            # 92-95/124-127) runs ~22 GB/s vs ~26 for the others, so home
            # 3 output rows on partitions 0:3 via a tiny 5th strip and
            # stop strip 3 at 125 rows -> queue 15 carries 29 row-slots
            # instead of 32 and stops being the straggler.
            if K128_SPLIT:
                strips = [(0, 128), (128, 128), (256, 128),
                          (384, 125), (509, 3)]
            else:
                strips = [(s * STRIP, STRIP) for s in range(N_STRIPS)]

            copy_ctr = 0
            for sb in range(NSB):
                n0 = sb * WSB
                w = WSB if sb < NSB - 1 else NC_F32 - n0  # 2304 last
                build_chunk(2 * sb)
                build_chunk(2 * sb + 1)
                stg = stage_pool.tile([128, len(strips) * 2304], f32,
                                      tag="stg")
                for si, (r0, h) in enumerate(strips):
                    lhs = lhsT_s[:, r0:r0 + h]
                    off = 0
                    while off < w:
                        pw = min(PAIR, w - off)
                        ps = psum_pool.tile([128, PAIR], f32, tag="ps")
                        o2 = 0
                        while o2 < pw:
                            mw = min(FULL, pw - o2)
                            nc.tensor.matmul(
                                ps[0:h, o2:o2 + mw],
                                lhs,
                                rhs_t[:, n0 + off + o2:n0 + off + o2 + mw],
                                start=True, stop=True,
                            )
                            o2 += mw
                        dst = stg[0:h, si * 2304 + off:si * 2304 + off + pw]
                        # mostly-ACT while DVE builds; 50/50 after
                        if (sb < 4 and copy_ctr % 4 != 0) or copy_ctr % 2:
                            nc.scalar.copy(out=dst, in_=ps[0:h, 0:pw])
                        else:
                            nc.vector.tensor_copy(out=dst, in_=ps[0:h, 0:pw])
                        copy_ctr += 1
                        off += pw
                    # sb 0: store halves so the stream starts ~2 us sooner
                    if sb == 0:
                        pieces = [(0, PAIR), (PAIR, w - PAIR)]
                    else:
                        pieces = [(0, w)]
                    for pi, (p0, plen) in enumerate(pieces):
                        eng = nc.sync if (sb + si + pi) % 2 == 0 else nc.scalar
                        eng.dma_start(
                            out=out_d[r0:r0 + h, n0 + p0:n0 + p0 + plen],
                            in_=stg[0:h,
                                    si * 2304 + p0:si * 2304 + p0 + plen],
                        )
    nc.compile()
    return nc


M_HY = 16 * B                 # 1024 rows per core (hybrid 4a x 2c shard)
N_HY = NC_F32 // 2            # 8320 f32 cols per core (one c-half)
NH_HY = N_HY // 2             # 4160 (packed rhs width)
W_HY = M_HY + NH_HY           # 5184 input width

N_CSH = NC_F32 // N_CORES     # 2080 f32 output cols per core (c-shard)
M_CSH = A * B                 # 4096 output rows per core (c-shard)
W_CSH = N_CSH + M_CSH // 2    # input width: rhs 2080 + half of lhsT 2048


def _build_program_cshard():
    """c-shard f32 variant: output cols n=(c,d,re/im) sharded 8 ways
    (16 c's per core), all (a,b) rows on every core.  M=4096, N=2080.
    Input [128, 4128] f32, rhs duplicated on both partition halves so
    strips 16..31 (lhsT packed on partitions 64:128) satisfy the matmul
    base-partition rule:
      rows 0:64   = [ rhs(2080) | lhsT[:, 0:2048]    ]   (strips 0..15)
      rows 64:128 = [ rhs(2080) | lhsT[:, 2048:4096] ]   (strips 16..31)
    """
    nc = bacc.Bacc("TRN2", target_bir_lowering=False, debug=False,
                   enable_asserts=False)
    f32 = mybir.dt.float32
    n_strips = M_CSH // STRIP  # 32

    inp_d = nc.dram_tensor("inp", [128, W_CSH], f32,
                           kind="ExternalInput").ap()
    out_d = nc.dram_tensor("out", [M_CSH, N_CSH], f32,
                           kind="ExternalOutput").ap()

    with tile.TileContext(nc) as tc:
        with (
            tc.tile_pool(name="const", bufs=1) as const_pool,
            tc.tile_pool(name="stage", bufs=4) as stage_pool,
            tc.tile_pool(name="psum", bufs=4, space="PSUM") as psum_pool,
        ):
            inp_s = const_pool.tile([128, W_CSH], f32)
            # chunked: rhs + strip0 lhsT first so compute starts early
            nc.sync.dma_start(out=inp_s[:, 0:N_CSH + STRIP],
                              in_=inp_d[:, 0:N_CSH + STRIP])
            nc.sync.dma_start(out=inp_s[:, N_CSH + STRIP:],
                              in_=inp_d[:, N_CSH + STRIP:])

            # Strip-pair (s, s+16) on disjoint PE row-groups (partitions
            # 0:64 vs 64:128): adjacent matmuls from the two groups run
            # CONCURRENTLY in the array (measured ~2x PE throughput), which
            # matters because fp32 matmuls cost 2 half-speed passes.
            lhs_lo = inp_s[0:64, N_CSH:]
            lhs_hi = inp_s[64:128, N_CSH:]
            rk_lo = inp_s[0:64, 0:N_CSH]
            rk_hi = inp_s[64:128, 0:N_CSH]

            copy_ctr = 0
            for sp in range(16):
                stage_a = stage_pool.tile([128, N_CSH], f32, tag="staging")
                stage_b = stage_pool.tile([128, N_CSH], f32, tag="staging")
                lhs_a = lhs_lo[:, sp * STRIP:(sp + 1) * STRIP]
                lhs_b = lhs_hi[:, sp * STRIP:(sp + 1) * STRIP]

                def do_block(n0, width):
                    """cols [n0, n0+width) of BOTH strips, interleaved."""
                    nonlocal copy_ctr
                    ps_a = psum_pool.tile([128, PAIR], f32, tag="ps")
                    ps_b = psum_pool.tile([128, PAIR], f32, tag="ps")
                    off = 0
                    while off < width:
                        w = min(FULL, width - off)
                        nc.tensor.matmul(
                            ps_a[:, off:off + w], lhs_a,
                            rk_lo[:, n0 + off:n0 + off + w],
                            start=True, stop=True)
                        nc.tensor.matmul(
                            ps_b[:, off:off + w], lhs_b,
                            rk_hi[:, n0 + off:n0 + off + w],
                            start=True, stop=True)
                        off += w
                    for ps, stg in ((ps_a, stage_a), (ps_b, stage_b)):
                        dst = stg[:, n0:n0 + width]
                        if copy_ctr % 4 == 3:
                            nc.scalar.copy(out=dst, in_=ps[:, 0:width])
                        else:
                            nc.vector.tensor_copy(out=dst, in_=ps[:, 0:width])
                        copy_ctr += 1

                do_block(0, PAIR)                     # 1024
                do_block(PAIR, PAIR)                  # 1024
                do_block(2 * PAIR, N_CSH - 2 * PAIR)  # 32

                nc.sync.dma_start(
                    out=out_d[sp * STRIP:(sp + 1) * STRIP, :],
                    in_=stage_a[:])
                nc.sync.dma_start(
                    out=out_d[(sp + 16) * STRIP:(sp + 17) * STRIP, :],
                    in_=stage_b[:])
    nc.compile()
    return nc


def _build_program_hybrid():
    """Hybrid shard: a split 4 ways (16 a-rows -> M=1024) x c split 2 ways
    (64 c's -> N=8320 f32).  Same structure as the a-shard f32 program but
    rhs is only replicated 2x across cores (2.65 MB input vs 4.52), and
    output HBM runs are still 33 KB/partition (near-peak store rate)."""
    nc = bacc.Bacc("TRN2", target_bir_lowering=False, debug=False,
                   enable_asserts=False)
    f32 = mybir.dt.float32
    n_strips = M_HY // STRIP  # 8

    inp_d = nc.dram_tensor("inp", [128, W_HY], f32, kind="ExternalInput").ap()
    out_d = nc.dram_tensor("out", [M_HY, N_HY], f32,
                           kind="ExternalOutput").ap()

    with tile.TileContext(nc) as tc:
        with (
            tc.tile_pool(name="const", bufs=1) as const_pool,
            tc.tile_pool(name="stage", bufs=3) as stage_pool,
            tc.tile_pool(name="psum", bufs=4, space="PSUM") as psum_pool,
        ):
            inp_s = const_pool.tile([128, W_HY], f32)
            bounds = [0, M_HY + 1024, M_HY + 2080, W_HY]
            for cidx in range(len(bounds) - 1):
                nc.scalar.dma_start(
                    out=inp_s[:, bounds[cidx]:bounds[cidx + 1]],
                    in_=inp_d[:, bounds[cidx]:bounds[cidx + 1]])
            lhsT_s = inp_s[:, 0:M_HY]
            rhs_s = inp_s[:, M_HY:W_HY]

            copy_ctr = 0
            for s in range(n_strips):
                staging = stage_pool.tile([128, N_HY], f32, tag="staging")
                lhs_lo = lhsT_s[0:K, s * STRIP:(s + 1) * STRIP]
                lhs_hi = lhsT_s[64:64 + K, s * STRIP:(s + 1) * STRIP]

                def do_block(n0, width, h):
                    nonlocal copy_ctr
                    ps = psum_pool.tile([128, PAIR], f32, tag="ps")
                    lhs = lhs_lo if h == 0 else lhs_hi
                    rk = rhs_s[0:K] if h == 0 else rhs_s[64:64 + K]
                    off = 0
                    while off < width:
                        w = min(FULL, width - off)
                        nc.tensor.matmul(
                            ps[:, off:off + w], lhs,
                            rk[:, n0 + off:n0 + off + w],
                            start=True, stop=True)
                        off += w
                    dst = staging[:, h * NH_HY + n0: h * NH_HY + n0 + width]
                    if copy_ctr % 2 == 1:
                        nc.scalar.copy(out=dst, in_=ps[:, 0:width])
                    else:
                        nc.vector.tensor_copy(out=dst, in_=ps[:, 0:width])
                    copy_ctr += 1

                for q in range(NH_HY // PAIR):       # 4 pairs per half
                    for h in (0, 1):
                        do_block(q * PAIR, PAIR, h)
                for h in (0, 1):                     # 64-col remainder
                    do_block((NH_HY // PAIR) * PAIR, NH_HY % PAIR, h)

                # completion-ordered pieces on the cold strips: halves of
                # each c-half-quarter finish at mid-strip (pair q1)
                if s < 3:
                    pieces = [(0, 2048), (4160, 6208),
                              (2048, 4160), (6208, N_HY)]
                else:
                    pieces = [(0, N_HY)]
                for oi, (c0, c1) in enumerate(pieces):
                    eng = nc.sync if (s + oi) % 2 == 0 else nc.scalar
                    eng.dma_start(
                        out=out_d[s * STRIP:(s + 1) * STRIP, c0:c1],
                        in_=staging[:, c0:c1],
                    )
    nc.compile()
    return nc


def _build_program():
    """Build the per-core Bass program (same NEFF on all 8 cores)."""
    nc = bacc.Bacc("TRN2", target_bir_lowering=False, debug=False,
                   enable_asserts=False)
    mdt = _mm_dtype()

    # lhsT2 and rhs fused into ONE input tensor/DMA: the fp32 self-loading
    # matmul (walrus S3_LW) only has a single sync-wait slot, so the first
    # matmul may depend on at most one DMA-completion semaphore lane.
    inp_d = nc.dram_tensor("inp", [128, M + N_HALF], mdt,
                           kind="ExternalInput").ap()
    out_d = nc.dram_tensor("out", [M, NC_F32], mybir.dt.float32,
                           kind="ExternalOutput").ap()

    f32 = mybir.dt.float32
    with tile.TileContext(nc) as tc:
        with (
            tc.tile_pool(name="const", bufs=1) as const_pool,
            tc.tile_pool(name="stage", bufs=2) as stage_pool,
            tc.tile_pool(name="psum", bufs=4, space="PSUM") as psum_pool,
        ):
            inp_s = const_pool.tile([128, M + N_HALF], mdt)
            # chunked input load on the ACT HWDGE ring (output stores use
            # the SP ring) with a small first chunk so matmuls start early
            bounds = [0, M + 1024, M + 3104, M + 5184, M + N_HALF]
            for cidx in range(len(bounds) - 1):
                nc.scalar.dma_start(
                    out=inp_s[:, bounds[cidx]:bounds[cidx + 1]],
                    in_=inp_d[:, bounds[cidx]:bounds[cidx + 1]])
            lhsT_s = inp_s[:, 0:M]
            rhs_s = inp_s[:, M:M + N_HALF]

            # HAM warm-up: dummy matmuls while the input DMA is in
            # flight, so real matmuls start at 2.4 GHz instead of 1.2
            if WARMUP:
                warm = const_pool.tile([64, 128], mdt)
                nc.vector.memset(warm[:], 0.0)
                ps_w = psum_pool.tile([128, PAIR], f32, tag="ps")
                for _ in range(WARMUP):
                    nc.tensor.matmul(ps_w[:, 0:128], warm[:], warm[:],
                                     start=True, stop=True)

            copy_ctr = 0
            for s in range(N_STRIPS):
                staging = stage_pool.tile([128, NC_F32], f32, tag="staging")
                lhs_lo = lhsT_s[0:K, s * STRIP:(s + 1) * STRIP]
                lhs_hi = lhsT_s[64:64 + K, s * STRIP:(s + 1) * STRIP]

                def do_block(n0, width, h):
                    """Matmul cols [n0, n0+width) of half h into psum, copy to
                    staging. width <= PAIR, split into FULL-sized matmuls."""
                    nonlocal copy_ctr
                    ps = psum_pool.tile([128, PAIR], f32, tag="ps")
                    lhs = lhs_lo if h == 0 else lhs_hi
                    rk = rhs_s[0:K] if h == 0 else rhs_s[64:64 + K]
                    off = 0
                    while off < width:
                        w = min(FULL, width - off)
                        nc.tensor.matmul(
                            ps[:, off:off + w],
                            lhs,
                            rk[:, n0 + off:n0 + off + w],
                            start=True, stop=True,
                        )
                        off += w
                    dst = staging[:, h * N_HALF + n0: h * N_HALF + n0 + width]
                    # 50/50 DVE/ACT split: combined evacuation ~920 GB/s,
                    # well above the ~410 GB/s store stream it feeds
                    if copy_ctr % 2 == 1:
                        nc.scalar.copy(out=dst, in_=ps[:, 0:width])
                    else:
                        nc.vector.tensor_copy(out=dst, in_=ps[:, 0:width])
                    copy_ctr += 1

                for q in range(N_PAIRS):
                    for h in (0, 1):
                        do_block(q * PAIR, PAIR, h)
                for h in (0, 1):
                    do_block(N_PAIRS * PAIR, REM, h)

                # Full-strip 8.5 MB stores run at ~411 GB/s vs ~350 for
                # smaller pieces.  With the left/right pair interleave,
                # output quarters [0:4160] and [8320:12480] are complete
                # at MID-strip, so for the early (cold-PE) strips issue
                # those first on opposite rings — the store stream keeps
                # flowing while the PE works through the rest of the
                # strip.  Later strips store whole; the final tail is
                # pure bandwidth backlog either way.
                if (OUT_SPLIT == "ordq" and s < 3) or s == 0:
                    pieces = [(0, 4160), (8320, 12480),
                              (4160, 8320), (12480, NC_F32)]
                else:
                    pieces = [(0, NC_F32)]
                for oi, (c0, c1) in enumerate(pieces):
                    eng = nc.sync if (s + oi) % 2 == 0 else nc.scalar
                    eng.dma_start(
                        out=out_d[s * STRIP:(s + 1) * STRIP, c0:c1],
                        in_=staging[:, c0:c1],
                    )
    nc.compile()
    return nc


def _get_program():
    global _PROGRAM
    if _PROGRAM is None:
        if VARIANT == "k128b":
            _PROGRAM = _build_program_k128b()
        elif VARIANT == "k128":
            _PROGRAM = _build_program_k128()
        elif VARIANT == "devrhs":
            _PROGRAM = _build_program_devrhs()
        elif VARIANT == "bf16":
            _PROGRAM = _build_program_bf16()
        elif VARIANT == "cshard":
            _PROGRAM = _build_program_cshard()
        elif VARIANT == "hybrid":
            _PROGRAM = _build_program_hybrid()
        else:
            _PROGRAM = _build_program()
    return _PROGRAM


def _as_complex(x):
    return x[..., 0].astype(np.complex64) + 1j * x[..., 1].astype(np.complex64)


def prepare_in_maps(diagonal_core, factor0, factor1, factor2, factor3):
    core = _as_complex(np.asarray(diagonal_core))    # [e]
    fa = _as_complex(np.asarray(factor0))            # [a, e]
    fb = _as_complex(np.asarray(factor1))            # [b, e]
    fc = _as_complex(np.asarray(factor2))            # [c, e]
    fd = _as_complex(np.asarray(factor3))            # [d, e]

    fa = fa * core[None, :]
    # B[n=(c,d), e] = fc[c,e] * fd[d,e]
    Bm = (fc[:, None, :] * fd[None, :, :]).reshape(C * D, RANK)   # [8320, 32]

    # rhs [K=64, 2*C*D] f32 with interleaved (re, im) output columns
    R = np.empty((K, 2 * C * D), dtype=np.float32)
    R[:RANK, 0::2] = Bm.real.T
    R[:RANK, 1::2] = Bm.imag.T
    R[RANK:, 0::2] = -Bm.imag.T
    R[RANK:, 1::2] = Bm.real.T

    if VARIANT == "devrhs":
        # tiny U/V outer-product factors instead of the 4.26 MB rhs.
        # packed[p, 2n'+t] = U[p,c']V[p,d] + U2[p,c']V2[p,d], groups of 32
        # partitions select (R-row-half, c-block); signs baked in.
        fcr, fci = fc.real.copy(), fc.imag.copy()        # [128, 32] f32
        fdr, fdi = fd.real.copy(), fd.imag.copy()        # [65, 32] f32
        uv = np.empty((128, W_UV), dtype=np.float32)
        for g in range(4):
            sl = slice(g * 32, (g + 1) * 32)
            cblk, rh = g // 2, g % 2
            fcr_g = fcr[cblk * 64:(cblk + 1) * 64, :].T  # [32e, 64c]
            fci_g = fci[cblk * 64:(cblk + 1) * 64, :].T
            fdr_g, fdi_g = fdr.T, fdi.T                  # [32e, 65d]
            if rh == 0:   # E=ReB, O=ImB
                ue1, ve1, ue2, ve2 = fcr_g, fdr_g, -fci_g, fdi_g
                uo1, vo1, uo2, vo2 = fcr_g, fdi_g, fci_g, fdr_g
            else:         # E=-ImB, O=ReB
                ue1, ve1, ue2, ve2 = -fcr_g, fdi_g, -fci_g, fdr_g
                uo1, vo1, uo2, vo2 = fcr_g, fdr_g, -fci_g, fdi_g
            uv[sl, 0:64], uv[sl, 64:128] = ue1, ue2
            uv[sl, 128:192], uv[sl, 192:256] = uo1, uo2
            uv[sl, 256:321], uv[sl, 321:386] = ve1, ve2
            uv[sl, 386:451], uv[sl, 451:516] = vo1, vo2
        in_maps = []
        for ci in range(N_CORES):
            fa_c = fa[ci * A_LOC:(ci + 1) * A_LOC]
            Am = (fa_c[:, None, :] * fb[None, :, :]).reshape(M, RANK)
            L = np.empty((K, M), dtype=np.float32)
            L[:RANK] = Am.real.T
            L[RANK:K] = Am.imag.T
            inp = np.empty((128, W_DEV), dtype=np.float32)
            inp[:, 0:W_UV] = uv
            inp[0:64, W_UV:] = L
            inp[64:128, W_UV:] = L
            in_maps.append({"inp": np.ascontiguousarray(inp)})
        return in_maps

    if VARIANT in ("k128", "k128b"):
        # K=128 pure-product rhs: 4 k-groups of 32 ranks pair lhsT rows
        # [ReA; ReA; ImA; ImA] with signed product planes so the PE's
        # K-reduction does the complex combines.  U2/V2 interleave the
        # even/odd (re/im output col) factor variants per c / d.
        fcr, fci = fc.real.copy(), fc.imag.copy()        # [128, 32]
        fdr, fdi = fd.real.copy(), fd.imag.copy()        # [65, 32]
        u2 = np.empty((128, W_U2), dtype=np.float32)
        v2 = np.empty((128, W_V2), dtype=np.float32)
        for g in range(4):
            sl = slice(g * 32, (g + 1) * 32)
            ue = [fcr, -fci, -fcr, -fci][g].T            # [32e, 128c]
            uo = [fcr, fci, fcr, -fci][g].T
            ve = [fdr, fdi, fdi, fdr][g].T               # [32e, 65d]
            vo = [fdi, fdr, fdr, fdi][g].T
            u2[sl, 0::2], u2[sl, 1::2] = ue, uo
            v2[sl, 0::2], v2[sl, 1::2] = ve, vo
        uvm = np.ascontiguousarray(np.concatenate([u2, v2], axis=1))
        in_maps = []
        for ci in range(N_CORES):
            fa_c = fa[ci * A_LOC:(ci + 1) * A_LOC]
            Am = (fa_c[:, None, :] * fb[None, :, :]).reshape(M, RANK)
            L = np.empty((128, M), dtype=np.float32)
            L[0:32] = Am.real.T
            L[32:64] = Am.real.T
            L[64:96] = Am.imag.T
            L[96:128] = Am.imag.T
            in_maps.append({"uv": uvm, "lhs": L})
        return in_maps

    if VARIANT == "hybrid":
        in_maps = []
        for ci in range(N_CORES):
            ai, hi = ci // 2, ci % 2
            fa_c = fa[ai * 16:(ai + 1) * 16]                      # [16, 32]
            Am = (fa_c[:, None, :] * fb[None, :, :]).reshape(M_HY, RANK)
            L = np.empty((K, M_HY), dtype=np.float32)
            L[:RANK] = Am.real.T
            L[RANK:] = Am.imag.T
            Rs = R[:, hi * N_HY:(hi + 1) * N_HY]                  # [64, 8320]
            inp = np.empty((128, W_HY), dtype=np.float32)
            inp[0:64, 0:M_HY] = L
            inp[64:128, 0:M_HY] = L
            inp[0:64, M_HY:] = Rs[:, 0:NH_HY]
            inp[64:128, M_HY:] = Rs[:, NH_HY:]
            in_maps.append({"inp": inp})
        return in_maps

    if VARIANT == "cshard":
        # all (a,b) rows on every core; (c,d) columns sharded
        Am = (fa[:, None, :] * fb[None, :, :]).reshape(A * B, RANK)
        L = np.empty((K, M_CSH), dtype=np.float32)
        L[:RANK] = Am.real.T
        L[RANK:] = Am.imag.T
        in_maps = []
        for ci in range(N_CORES):
            Rs = R[:, ci * N_CSH:(ci + 1) * N_CSH]       # [64, 2080]
            inp = np.empty((128, W_CSH), dtype=np.float32)
            inp[0:64, 0:N_CSH] = Rs
            inp[64:128, 0:N_CSH] = Rs
            inp[0:64, N_CSH:] = L[:, 0:M_CSH // 2]
            inp[64:128, N_CSH:] = L[:, M_CSH // 2:]
            in_maps.append({"inp": inp})
        return in_maps

    if VARIANT == "bf16":
        import ml_dtypes
        bf = ml_dtypes.bfloat16
        Rh = R.astype(bf)
        Rl = (R - Rh.astype(np.float32)).astype(bf)
        rhs1 = np.concatenate([Rh, Rl], axis=0)          # [128, 16640] bf16
    else:
        # pack column halves onto partition halves -> [128, 8320]
        rhs_packed = np.ascontiguousarray(
            np.concatenate([R[:, :N_HALF], R[:, N_HALF:]], axis=0))

    in_maps = []
    for ci in range(N_CORES):
        fa_c = fa[ci * A_LOC:(ci + 1) * A_LOC]                    # [8, 32]
        Am = (fa_c[:, None, :] * fb[None, :, :]).reshape(M, RANK)  # [512, 32]
        L = np.empty((K, M), dtype=np.float32)
        L[:RANK] = Am.real.T
        L[RANK:K] = Am.imag.T
        if VARIANT == "bf16":
            Lh = L.astype(bf)
            Ll = (L - Lh.astype(np.float32)).astype(bf)
            lhsT1 = np.concatenate([Lh, Lh], axis=0)     # [128, 512]
            lhsT2 = np.concatenate([Ll, np.zeros_like(Ll)], axis=0)
            inp = np.concatenate([lhsT1, lhsT2, rhs1], axis=1)
        else:
            L2 = np.concatenate([L, L], axis=0)  # duplicate for row-group 64
            inp = np.concatenate([L2, rhs_packed], axis=1)
        in_maps.append({"inp": np.ascontiguousarray(inp)})
    return in_maps


def kernel(diagonal_core, factor0, factor1, factor2, factor3):
    global LAST_RESULTS
    if TRACE:
        _install_trace_shim()
    in_maps = prepare_in_maps(diagonal_core, factor0, factor1,
                              factor2, factor3)
    nc = _get_program()
    res = run_bass_kernel_spmd(nc, in_maps, core_ids=list(range(N_CORES)),
                               trace=TRACE)
    LAST_RESULTS = res

    out = np.empty((A, B, C, D), dtype=np.complex64)
    c_sh = C // N_CORES  # 16
    for ci in range(N_CORES):
        part = res.results[ci]["out"]
        if VARIANT == "hybrid":
            ai, hi = ci // 2, ci % 2
            out[ai * 16:(ai + 1) * 16, :, hi * 64:(hi + 1) * 64, :] = (
                part.reshape(16, B, 64, D, 2).view(np.complex64)
                .squeeze(-1))
        elif VARIANT == "cshard":
            out[:, :, ci * c_sh:(ci + 1) * c_sh, :] = (
                part.reshape(A, B, c_sh, D, 2).view(np.complex64)
                .squeeze(-1))
        else:
            out[ci * A_LOC:(ci + 1) * A_LOC] = (
                part.reshape(A_LOC, B, C, D, 2).view(np.complex64)
                .squeeze(-1))
    return out

